# revision 1
# baseline (speedup 1.0000x reference)
"""Distributed causal multi-head attention block for 8 TRN2 NeuronCores.

Sharding: core i -> (batch b = i//2, head-half hh = i%2).  Each core computes
attention for 6 of the 12 heads of one batch element, then a row-sharded
c_proj (its 384 input channels -> full 768 outputs, partial sums).  The
host sums the two partial projections per batch (the "all-reduce" of the
tensor-parallel c_proj) and adds b_proj.

Everything on-chip lives transposed ([feature, token]) so no transposes are
needed:
  qkT = W_qk @ x^T          (heads' Q^T,K^T in [d, t] layout)
  V   = x @ Wv^T            ([t, d] layout, + per-head ones column)
  S^T = K_blk @ Q_blk^T     -> exp (scale 1/8 folded in) -> causal mask
  O^T_aug = [V|1]^T @ P^T   (row 64 of each head block = softmax denom)
  y^T = O^T * (1/denom)  + bv
  out^T = Wp_half @ y^T     (partial over this core's heads)
"""

import sys

sys.path.insert(0, "/opt/trn_rl_repo")

import numpy as np
import ml_dtypes

import concourse.bass as bass
import concourse.bacc as bacc
import concourse.mybir as mybir
import concourse.tile as tile
from concourse.bass_utils import run_bass_kernel_spmd

BF16 = mybir.dt.bfloat16
F32 = mybir.dt.float32
F32R = mybir.dt.float32r
AF = mybir.ActivationFunctionType
ALU = mybir.AluOpType

B, T, C, H, HD = 4, 2048, 768, 12, 64
NCORES = 8
HH = 6              # heads per core
CH = HH * HD        # 384 channels per core
NCT = C // 128      # 6 contraction tiles over C
NTT = T // 128      # 16 token tiles
NQC = T // 512      # 4 query chunks
VW = 65             # per-head V block width (64 dims + ones column)


def _build_graph():
    nc = bacc.Bacc("TRN2", target_bir_lowering=False)

    xT = nc.declare_dram_parameter("xT", [C, T], BF16, isOutput=False)
    wqkT = nc.declare_dram_parameter("wqkT", [C, 2 * CH], BF16, isOutput=False)
    bqk = nc.declare_dram_parameter("bqk", [128, 2 * CH // 128], F32, isOutput=False)
    wvT = nc.declare_dram_parameter("wvT", [C, CH], BF16, isOutput=False)
    bv = nc.declare_dram_parameter("bv", [128, CH // 128], F32, isOutput=False)
    wpT = nc.declare_dram_parameter("wpT", [CH, C], BF16, isOutput=False)
    masks = nc.declare_dram_parameter("masks", [128, 4 * 1024], BF16, isOutput=False)
    out = nc.declare_dram_parameter("out", [C, T], F32, isOutput=True)

    with tile.TileContext(nc) as tc:
        with (
            tc.tile_pool(name="weights", bufs=1) as wpool,
            tc.tile_pool(name="acts", bufs=1) as apool,
            tc.tile_pool(name="ps2", bufs=2, space="PSUM") as ps2,
            tc.tile_pool(name="pacc", bufs=2, space="PSUM") as pacc,
            tc.tile_pool(name="ptile", bufs=8) as ppool,
            tc.tile_pool(name="small", bufs=6) as spool,
            tc.tile_pool(name="ostage", bufs=6) as opool,
        ):
            # ---- load everything ----
            xT_s = [wpool.tile([128, T], BF16, tag=f"xT{i}", name=f"xT{i}") for i in range(NCT)]
            wqkT_s = [wpool.tile([128, 2 * CH], BF16, tag=f"wqk{i}", name=f"wqk{i}") for i in range(NCT)]
            wvT_s = [wpool.tile([128, CH], BF16, tag=f"wv{i}", name=f"wv{i}") for i in range(NCT)]
            wpT_s = [wpool.tile([128, C], BF16, tag=f"wp{i}", name=f"wp{i}") for i in range(CH // 128)]
            bqk_s = wpool.tile([128, 2 * CH // 128], F32, tag="bqk")
            bv_s = wpool.tile([128, CH // 128], F32, tag="bv")
            masks_s = wpool.tile([128, 4 * 1024], BF16, tag="masks")
            # first QK-proj accumulation chain consumes (wqkT[ct], xT[ct]) in
            # ct order; interleave the loads (first 512-token slice of each
            # xT tile first) so PE can start a couple of us in instead of
            # after the whole load phase
            nc.sync.dma_start(bqk_s[:], bqk[:, :])
            for i in range(NCT):
                nc.scalar.dma_start(wqkT_s[i][:], wqkT[i * 128:(i + 1) * 128, :])
                nc.sync.dma_start(xT_s[i][:, 0:1024], xT[i * 128:(i + 1) * 128, 0:1024])
            for i in range(NCT):
                (nc.sync if i % 2 else nc.scalar).dma_start(
                    xT_s[i][:, 1024:], xT[i * 128:(i + 1) * 128, 1024:])
            for i in range(NCT):
                (nc.scalar if i % 2 else nc.sync).dma_start(
                    wvT_s[i][:], wvT[i * 128:(i + 1) * 128, :])
            nc.scalar.dma_start(masks_s[:], masks[:, :])
            nc.sync.dma_start(bv_s[:], bv[:, :])
            for i in range(CH // 128):
                nc.sync.dma_start(wpT_s[i][:], wpT[i * 128:(i + 1) * 128, :])
            # pre-touch bqk on DVE early (single-wait discipline for the
            # first bias copy); bv/masks are touched late enough naturally
            scratch = wpool.tile([128, 4], F32, tag="scratch")
            nc.vector.tensor_copy(scratch[:, 0:1], bqk_s[:, 0:1])

            # qkT rows: tiles 0..2 = Q^T (384 rows), 3..5 = K^T
            qkT_s = [apool.tile([128, T], BF16, tag=f"qkT{i}", name=f"qkT{i}") for i in range(NCT)]
            v_s = [apool.tile([128, HH * VW], BF16, tag=f"v{i}", name=f"v{i}") for i in range(NTT)]
            yT_s = [apool.tile([128, T], BF16, tag=f"yT{i}", name=f"yT{i}") for i in range(CH // 128)]

            # ---- QK^T projection (emitted per head-pair, interleaved
            # with attention so ACT starts early) ----
            def qk_proj(ot):
                # ot 0/3 run before any attention: fuse two token chunks over
                # a 2-bank PSUM so one ACT copy covers both.  The later tiles
                # interleave with attention, where the s2-tag slots are
                # contended -- keep those on single-bank "mm" slots.
                if ot in (0, 3):
                    for tc2 in range(NQC // 2):
                        ps = ps2.tile([128, 1024], F32, tag="s2", name="ps")
                        for ct in range(NCT):
                            for half in range(2):
                                nc.tensor.matmul(
                                    ps[:, half * 512:(half + 1) * 512],
                                    lhsT=wqkT_s[ct][:, ot * 128:(ot + 1) * 128],
                                    rhs=xT_s[ct][:, (tc2 * 2 + half) * 512:
                                                (tc2 * 2 + half + 1) * 512],
                                    start=(ct == 0),
                                    stop=(ct == NCT - 1),
                                )
                        nc.scalar.activation(
                            qkT_s[ot][:, tc2 * 1024:(tc2 + 1) * 1024], ps[:],
                            AF.Identity, bias=bqk_s[:, ot:ot + 1], scale=1.0,
                        )
                    return
                for tcn in range(NQC):
                    ps = ps2.tile([128, 512], F32, tag="mm", name="ps")
                    for ct in range(NCT):
                        nc.tensor.matmul(
                            ps[:],
                            lhsT=wqkT_s[ct][:, ot * 128:(ot + 1) * 128],
                            rhs=xT_s[ct][:, tcn * 512:(tcn + 1) * 512],
                            start=(ct == 0),
                            stop=(ct == NCT - 1),
                        )
                    nc.scalar.activation(
                        qkT_s[ot][:, tcn * 512:(tcn + 1) * 512], ps[:],
                        AF.Identity, bias=bqk_s[:, ot:ot + 1], scale=1.0,
                    )

            def v_proj(tt):
                ps = ps2.tile([128, CH], F32, tag="mm", name="ps")
                for ct in range(NCT):
                    nc.tensor.matmul(
                        ps[:],
                        lhsT=xT_s[ct][:, tt * 128:(tt + 1) * 128],
                        rhs=wvT_s[ct][:],
                        start=(ct == 0),
                        stop=(ct == NCT - 1),
                    )
                v3 = v_s[tt][:].rearrange("p (h w) -> p h w", w=VW)
                nc.scalar.copy(
                    v3[:, :, 0:64], ps[:].rearrange("p (h d) -> p h d", d=64)
                )
                nc.vector.memset(v3[:, :, 64:65], 1.0)

            def proj_out(tcn):
                for ot in range(NCT):
                    ps = ps2.tile([128, 512], F32, tag="mm", name="ps")
                    for ct in range(CH // 128):
                        nc.tensor.matmul(
                            ps[:],
                            lhsT=wpT_s[ct][:, ot * 128:(ot + 1) * 128],
                            rhs=yT_s[ct][:, tcn * 512:(tcn + 1) * 512],
                            start=(ct == 0),
                            stop=(ct == CH // 128 - 1),
                        )
                    so = opool.tile([128, 512], F32, tag="so", name="so")
                    nc.scalar.copy(so[:], ps[:])
                    nc.sync.dma_start(
                        out[ot * 128:(ot + 1) * 128, tcn * 512:(tcn + 1) * 512],
                        so[:],
                    )

            def attention_block(hp, qc):
                    qt = hp      # Q^T rows for heads 2hp,2hp+1 live in tile hp
                    ktile = 3 + hp
                    o_acc = [pacc.tile([65, 512], F32, tag="oacc", name="oacc")
                             for _ in range(2)]
                    nkt = 4 * (qc + 1)
                    pend = None  # software pipeline: O(kt) issued after S(kt+1)

                    def emit_o(p2, kt, w):
                        for hi in range(2):
                            h = 2 * hp + hi
                            nc.tensor.matmul(
                                o_acc[hi][:, w:512],
                                lhsT=v_s[kt][:, h * VW:(h + 1) * VW],
                                rhs=p2[:, hi * 512 + w:(hi + 1) * 512],
                                start=(kt == 0),
                                stop=(kt == nkt - 1),
                            )

                    for kt in range(nkt):
                        j = kt - 4 * qc
                        # columns q < j*128 of a diagonal block are fully
                        # masked: S, exp, mask and O all skip them (the first
                        # O matmul, start=True, is always full width)
                        w = j * 128 if j >= 1 else 0
                        s2 = ps2.tile([128, 1024], F32, tag="s2", name="s2")
                        for hi in range(2):
                            base = hi * 64
                            nc.tensor.matmul(
                                s2[:, hi * 512 + w:(hi + 1) * 512],
                                lhsT=qkT_s[ktile][base:base + 64,
                                                  kt * 128:(kt + 1) * 128],
                                rhs=qkT_s[qt][base:base + 64,
                                              qc * 512 + w:(qc + 1) * 512],
                                start=True, stop=True,
                            )
                        if pend is not None:
                            emit_o(*pend)
                        p2 = ppool.tile([128, 1024], BF16, tag="pt", name="p2")
                        p3 = p2[:].rearrange("p (c q) -> p c q", c=2)
                        s3 = s2[:].rearrange("p (c q) -> p c q", c=2)
                        nc.scalar.activation(
                            p3[:, :, w:512], s3[:, :, w:512], AF.Exp,
                            scale=0.125,
                        )
                        if j >= 0:
                            m3 = masks_s[:, j * 1024:(j + 1) * 1024].rearrange(
                                "p (c q) -> p c q", c=2)
                            nc.vector.tensor_mul(
                                p3[:, :, w:512], p3[:, :, w:512], m3[:, :, w:512]
                            )
                        pend = (p2, kt, w)
                    emit_o(*pend)
                    # normalize -> y^T; copy accumulator out first (single
                    # DVE op) so the PSUM bank frees immediately
                    for hi in range(2):
                        base = hi * 64
                        dn = spool.tile([1, 512], F32, tag="dn", name="dn")
                        nc.vector.tensor_copy(dn[:], o_acc[hi][64:65, :])
                        ob = spool.tile([64, 512], F32, tag="ob", name="ob")
                        nc.vector.tensor_copy(ob[:], o_acc[hi][0:64, :])
                        rn = spool.tile([1, 512], F32, tag="rn", name="rn")
                        nc.vector.reciprocal_approx_fast(rn[:], dn[:])
                        rc = spool.tile([64, 512], F32, tag="rc", name="rc")
                        nc.gpsimd.partition_broadcast(rc[:], rn[:], channels=64)
                        ysl = yT_s[hp][base:base + 64, qc * 512:(qc + 1) * 512]
                        nc.vector.tensor_mul(ysl, ob[:], rc[:])
                        nc.vector.tensor_scalar_add(
                            ysl, ysl, bv_s[base:base + 64, hp:hp + 1]
                        )

            qk_proj(0)
            qk_proj(3)
            for qc in range(NQC):
                for tt in range(4 * qc, 4 * qc + 4):
                    v_proj(tt)
                attention_block(0, qc)
            qk_proj(1)
            qk_proj(4)
            for qc in range(NQC):
                attention_block(1, qc)
            qk_proj(2)
            qk_proj(5)
            for qc in (3, 2, 1, 0):
                attention_block(2, qc)
                proj_out(qc)
    nc.compile()
    return nc


_CACHE: dict = {}


def _get_graph():
    if "nc" not in _CACHE:
        _CACHE["nc"] = _build_graph()
    return _CACHE["nc"]


def _bf16(a):
    return np.ascontiguousarray(a.astype(ml_dtypes.bfloat16))


def _make_masks():
    k = np.arange(128)[:, None]
    q = np.arange(512)[None, :]
    m = np.zeros((128, 4 * 1024), np.float32)
    for j in range(4):
        pat = (q >= k + j * 128).astype(np.float32)
        m[:, j * 1024:j * 1024 + 512] = pat
        m[:, j * 1024 + 512:(j + 1) * 1024] = pat
    return _bf16(m)


def _prepare_in_maps(x, W_attn, b_attn, W_proj):
    masks = _make_masks()
    in_maps = []
    for core in range(NCORES):
        b, hh = core // 2, core % 2
        sl = slice(hh * CH, (hh + 1) * CH)
        wq = W_attn[0 * C:1 * C][sl]          # [384, 768]
        wk = W_attn[1 * C:2 * C][sl]
        wv = W_attn[2 * C:3 * C][sl]
        bq = b_attn[0 * C:1 * C][sl]
        bk = b_attn[1 * C:2 * C][sl]
        bvv = b_attn[2 * C:3 * C][sl]
        in_maps.append({
            "xT": _bf16(x[b].T),                                   # [768, 2048]
            "wqkT": _bf16(np.concatenate([wq, wk], 0).T),          # [768, 768]
            "bqk": np.ascontiguousarray(
                np.concatenate([bq, bk]).reshape(-1, 128).T),      # [128, 6]
            "wvT": _bf16(wv.T),                                    # [768, 384]
            "bv": np.ascontiguousarray(bvv.reshape(-1, 128).T),    # [128, 3]
            "wpT": _bf16(W_proj[:, sl].T),                         # [384, 768]
            "masks": masks,
        })
    return in_maps


def _unshard(outs, b_proj):
    y = np.empty((B, T, C), np.float32)
    for b in range(B):
        y[b] = (outs[2 * b]["out"] + outs[2 * b + 1]["out"]).T + b_proj
    return y


def run(x, W_attn, b_attn, W_proj, b_proj, **spmd_kwargs):
    x = np.asarray(x, np.float32)
    W_attn = np.asarray(W_attn, np.float32)
    b_attn = np.asarray(b_attn, np.float32)
    W_proj = np.asarray(W_proj, np.float32)
    b_proj = np.asarray(b_proj, np.float32)
    in_maps = _prepare_in_maps(x, W_attn, b_attn, W_proj)
    nc = _get_graph()
    res = run_bass_kernel_spmd(
        nc, in_maps, core_ids=list(range(NCORES)), **spmd_kwargs
    )
    return _unshard(res.results, b_proj), res


def kernel(x, W_attn, b_attn, W_proj, b_proj):
    y, _ = run(x, W_attn, b_attn, W_proj, b_proj)
    return y



# revision 7
# speedup vs baseline: 1.0818x; 1.0818x over previous
"""Distributed causal multi-head attention block for 8 TRN2 NeuronCores.

Sharding: core i -> (batch b = i//2, head-half hh = i%2).  Each core computes
attention for 6 of the 12 heads of one batch element, then a row-sharded
c_proj (its 384 input channels -> full 768 outputs, partial sums).  The
host sums the two partial projections per batch and adds the constant
vector (b_proj + W_proj @ b_v); softmax rows sum to 1 so the v-bias
contributes exactly +b_v per channel and commutes through c_proj.

Engine plan (per core):
  ACT (scalar) : exp ONLY (the softmax exponentials are the wall --
                 ~12.6M elements at 1 elem/cycle/lane @1.2GHz)
  PE  (tensor) : qkv proj, S^T = K@Q^T (two heads row-tiled concurrently
                 via base-partition 0/64), O^T = [V|1]^T@P^T, c_proj
  DVE (vector) : qk bias add, V copy, causal mask (128-wide diagonal
                 square only), softmax normalize, c_proj PSUM->SBUF copy
  GPSIMD       : partition-broadcast of the softmax reciprocal

All projection matmuls are emitted as micro-fillers (<=3 matmuls each)
interleaved one-per-kt into the attention loop so the strict-FIFO PE
queue fills its gaps while ACT streams exps back-to-back.

All per-tensor SBUF residents use a packed [128, n_tiles*W] layout so
each load is one large contiguous DMA (host pre-transposes accordingly).
"""

import sys

sys.path.insert(0, "/opt/trn_rl_repo")

import numpy as np
import ml_dtypes

import concourse.bass as bass
import concourse.bacc as bacc
import concourse.mybir as mybir
import concourse.tile as tile
from concourse.bass_utils import run_bass_kernel_spmd

BF16 = mybir.dt.bfloat16
F32 = mybir.dt.float32
AF = mybir.ActivationFunctionType

B, T, C, H, HD = 4, 2048, 768, 12, 64
NCORES = 8
HH = 6              # heads per core
CH = HH * HD        # 384 channels per core
NCT = C // 128      # 6 contraction tiles over C
NQC = T // 512      # 4 query chunks
VW = 65             # per-head V block width (64 dims + ones column)


def _build_graph():
    nc = bacc.Bacc("TRN2", target_bir_lowering=False)

    # packed layouts: [128, n_tiles * width], tile ct at cols [ct*W,(ct+1)*W)
    xT = nc.declare_dram_parameter("xT", [128, NCT * T], BF16, isOutput=False)
    wqkT = nc.declare_dram_parameter("wqkT", [128, NCT * 2 * CH], BF16, isOutput=False)
    bqk = nc.declare_dram_parameter("bqk", [128, 2 * CH // 128], F32, isOutput=False)
    wvT = nc.declare_dram_parameter("wvT", [128, NCT * CH], BF16, isOutput=False)
    wpT = nc.declare_dram_parameter("wpT", [128, (CH // 128) * C], BF16, isOutput=False)
    tri = nc.declare_dram_parameter("tri", [128, 256], BF16, isOutput=False)
    out = nc.declare_dram_parameter("out", [C, T], BF16, isOutput=True)

    with tile.TileContext(nc) as tc:
        with (
            tc.tile_pool(name="weights", bufs=1) as wpool,
            tc.tile_pool(name="acts", bufs=1) as apool,
            tc.tile_pool(name="ps2", bufs=2, space="PSUM") as ps2,
            tc.tile_pool(name="pacc", bufs=2, space="PSUM") as pacc,
            tc.tile_pool(name="pmm", bufs=2, space="PSUM") as pmm,
            tc.tile_pool(name="ptile", bufs=8) as ppool,
            tc.tile_pool(name="small", bufs=6) as spool,
            tc.tile_pool(name="ostage", bufs=6) as opool,
        ):
            xT_b = wpool.tile([128, NCT * T], BF16, tag="xT")
            wqkT_b = wpool.tile([128, NCT * 2 * CH], BF16, tag="wqk")
            wvT_b = wpool.tile([128, NCT * CH], BF16, tag="wv")
            wpT_b = wpool.tile([128, (CH // 128) * C], BF16, tag="wp")
            bqk_s = wpool.tile([128, 2 * CH // 128], F32, tag="bqk")
            tri_s = wpool.tile([128, 256], BF16, tag="tri")

            def xts(ct):
                return xT_b[:, ct * T:(ct + 1) * T]

            def wqks(ct):
                return wqkT_b[:, ct * 2 * CH:(ct + 1) * 2 * CH]

            def wvs(ct):
                return wvT_b[:, ct * CH:(ct + 1) * CH]

            def wps(ct):
                return wpT_b[:, ct * C:(ct + 1) * C]

            qkT_s = [apool.tile([128, T], BF16, tag=f"qkT{i}", name=f"qkT{i}") for i in range(NCT)]
            v_s = [apool.tile([128, HH * VW], BF16, tag=f"v{i}", name=f"v{i}") for i in range(16)]
            yT_s = [apool.tile([128, T], BF16, tag=f"yT{i}", name=f"yT{i}") for i in range(CH // 128)]

            # ---- loads: one large contiguous DMA per tensor, critical
            # path (wqkT + first xT column chunk) first, queues balanced ----
            x3d_src = xT[:].rearrange("p (c t) -> p c t", c=NCT)
            x3d_dst = xT_b[:].rearrange("p (c t) -> p c t", c=NCT)
            nc.sync.dma_start(bqk_s[:], bqk[:, :])
            nc.scalar.dma_start(tri_s[:], tri[:, :])
            nc.sync.dma_start(wqkT_b[:], wqkT[:, :])
            nc.scalar.dma_start(x3d_dst[:, :, 0:512], x3d_src[:, :, 0:512])
            nc.scalar.dma_start(wvT_b[:], wvT[:, :])
            nc.sync.dma_start(x3d_dst[:, :, 512:1024], x3d_src[:, :, 512:1024])
            nc.scalar.dma_start(x3d_dst[:, :, 1024:1536], x3d_src[:, :, 1024:1536])
            nc.sync.dma_start(x3d_dst[:, :, 1536:2048], x3d_src[:, :, 1536:2048])
            nc.scalar.dma_start(wpT_b[:], wpT[:, :])

            # ones columns of the V blocks never change: set them once
            for tt in range(16):
                v3 = v_s[tt][:].rearrange("p (h w) -> p h w", w=VW)
                nc.vector.memset(v3[:, :, 64:65], 1.0)

            tri3 = tri_s[:].rearrange("p (c q) -> p c q", c=2)

            # ---- micro-fillers: <=3 matmuls each, popped one per kt ----
            fillers = []

            def pop_filler():
                if fillers:
                    fillers.pop(0)()

            def flush_fillers():
                while fillers:
                    fillers.pop(0)()

            def qk_micros(ot, tcn):
                st = {}

                def m1(ot=ot, tcn=tcn):
                    ps = pmm.tile([128, 512], F32, tag="mm", name="ps")
                    st["ps"] = ps
                    for ct in range(3):
                        nc.tensor.matmul(
                            ps[:],
                            lhsT=wqks(ct)[:, ot * 128:(ot + 1) * 128],
                            rhs=xts(ct)[:, tcn * 512:(tcn + 1) * 512],
                            start=(ct == 0), stop=False,
                        )

                def m2(ot=ot, tcn=tcn):
                    ps = st["ps"]
                    for ct in range(3, NCT):
                        nc.tensor.matmul(
                            ps[:],
                            lhsT=wqks(ct)[:, ot * 128:(ot + 1) * 128],
                            rhs=xts(ct)[:, tcn * 512:(tcn + 1) * 512],
                            start=False, stop=(ct == NCT - 1),
                        )
                    nc.vector.tensor_scalar_add(
                        qkT_s[ot][:, tcn * 512:(tcn + 1) * 512], ps[:],
                        bqk_s[:, ot:ot + 1],
                    )

                return [m1, m2]

            def v_micros(tt):
                st = {}

                def m1(tt=tt):
                    ps = pmm.tile([128, CH], F32, tag="mm", name="ps")
                    st["ps"] = ps
                    for ct in range(3):
                        nc.tensor.matmul(
                            ps[:],
                            lhsT=xts(ct)[:, tt * 128:(tt + 1) * 128],
                            rhs=wvs(ct)[:],
                            start=(ct == 0), stop=False,
                        )

                def m2(tt=tt):
                    ps = st["ps"]
                    for ct in range(3, NCT):
                        nc.tensor.matmul(
                            ps[:],
                            lhsT=xts(ct)[:, tt * 128:(tt + 1) * 128],
                            rhs=wvs(ct)[:],
                            start=False, stop=(ct == NCT - 1),
                        )
                    v3 = v_s[tt][:].rearrange("p (h w) -> p h w", w=VW)
                    nc.vector.tensor_copy(
                        v3[:, :, 0:64], ps[:].rearrange("p (h d) -> p h d", d=64)
                    )

                return [m1, m2]

            def proj_micro(ot, tcn):
                def m(ot=ot, tcn=tcn):
                    ps = pmm.tile([128, 512], F32, tag="mm", name="ps")
                    for ct in range(CH // 128):
                        nc.tensor.matmul(
                            ps[:],
                            lhsT=wps(ct)[:, ot * 128:(ot + 1) * 128],
                            rhs=yT_s[ct][:, tcn * 512:(tcn + 1) * 512],
                            start=(ct == 0),
                            stop=(ct == CH // 128 - 1),
                        )
                    so = opool.tile([128, 512], BF16, tag="so", name="so")
                    nc.vector.tensor_copy(so[:], ps[:])
                    nc.sync.dma_start(
                        out[ot * 128:(ot + 1) * 128, tcn * 512:(tcn + 1) * 512],
                        so[:],
                    )
                return [m]

            def attention_block(hp, qc, pops_per_kt=1):
                qt = hp          # Q^T rows for heads 2hp,2hp+1 live in tile hp
                ktile = 3 + hp
                o_acc = [pacc.tile([65, 512], F32, tag="oacc", name="oacc")
                         for _ in range(2)]
                nkt = 4 * (qc + 1)
                pend = []        # software pipeline: O(kt) issued after S(kt+2)

                def emit_o(p2, kt, w):
                    for hi in range(2):
                        h = 2 * hp + hi
                        nc.tensor.matmul(
                            o_acc[hi][:, w:512],
                            lhsT=v_s[kt][:, h * VW:(h + 1) * VW],
                            rhs=p2[:, hi * 512 + w:(hi + 1) * 512],
                            start=(kt == 0),
                            stop=(kt == nkt - 1),
                        )

                for kt in range(nkt):
                    j = kt - 4 * qc
                    # columns q < j*128 of a diagonal block are fully
                    # masked: S, exp and O all skip them (the first O
                    # matmul, start=True, is always full width)
                    w = j * 128 if j >= 1 else 0
                    s2 = ps2.tile([128, 1024], F32, tag="s2", name="s2")
                    for hi in range(2):
                        base = hi * 64
                        nc.tensor.matmul(
                            s2[:, hi * 512 + w:(hi + 1) * 512],
                            lhsT=qkT_s[ktile][base:base + 64,
                                              kt * 128:(kt + 1) * 128],
                            rhs=qkT_s[qt][base:base + 64,
                                          qc * 512 + w:(qc + 1) * 512],
                            start=True, stop=True,
                        )
                    if len(pend) >= 2:
                        emit_o(*pend.pop(0))
                    p2 = ppool.tile([128, 1024], BF16, tag="pt", name="p2")
                    p3 = p2[:].rearrange("p (c q) -> p c q", c=2)
                    s3 = s2[:].rearrange("p (c q) -> p c q", c=2)
                    nc.scalar.activation(
                        p3[:, :, w:512], s3[:, :, w:512], AF.Exp, scale=0.125,
                    )
                    if j >= 0:
                        # only the 128-wide diagonal square is partially
                        # masked; its pattern is the same triangle for all j
                        nc.vector.tensor_mul(
                            p3[:, :, j * 128:(j + 1) * 128],
                            p3[:, :, j * 128:(j + 1) * 128],
                            tri3[:, :, :],
                        )
                    pend.append((p2, kt, w))
                    for _ in range(pops_per_kt):
                        pop_filler()
                # remaining fillers may include V copies that the trailing
                # O matmuls read: flush them first
                flush_fillers()
                while pend:
                    emit_o(*pend.pop(0))
                # normalize -> y^T (unbiased; v-bias folded into host add).
                # the denominator row is copied to a partition-0 SBUF tile
                # first: the custom-DVE reciprocal must read partition-
                # aligned SBUF on hardware.
                for hi in range(2):
                    base = hi * 64
                    dn = spool.tile([1, 512], F32, tag="dn", name="dn")
                    nc.vector.tensor_copy(dn[:], o_acc[hi][64:65, :])
                    rn = spool.tile([1, 512], F32, tag="rn", name="rn")
                    nc.vector.reciprocal_approx_fast(rn[:], dn[:])
                    rc = spool.tile([64, 512], F32, tag="rc", name="rc")
                    nc.gpsimd.partition_broadcast(rc[:], rn[:], channels=64)
                    ysl = yT_s[hp][base:base + 64, qc * 512:(qc + 1) * 512]
                    nc.vector.tensor_mul(ysl, o_acc[hi][0:64, :], rc[:])

            # ---- emission schedule (engine queues are strict FIFO, so
            # every filler a block's first instructions depend on must be
            # flushed before the block starts; fillers inside a block may
            # only depend on data ready by their pop slot) ----
            for m in qk_micros(0, 0) + qk_micros(3, 0):
                m()
            fillers += v_micros(0) + v_micros(1) + v_micros(2) + v_micros(3)
            fillers += qk_micros(0, 1)
            attention_block(0, 0, pops_per_kt=2)     # 4 kts x 2 pops
            fillers += qk_micros(3, 1) + v_micros(4) + v_micros(5) \
                + v_micros(6) + v_micros(7) + qk_micros(0, 2)
            attention_block(0, 1)            # 8 kts
            fillers += qk_micros(3, 2) + v_micros(8) + v_micros(9) \
                + v_micros(10) + v_micros(11) + qk_micros(0, 3)
            attention_block(0, 2)            # 12 kts
            fillers += qk_micros(3, 3) + v_micros(12) + v_micros(13) \
                + v_micros(14) + v_micros(15) + qk_micros(1, 0) + qk_micros(4, 0)
            attention_block(0, 3)            # 16 kts
            fillers += qk_micros(1, 1) + qk_micros(4, 1)
            attention_block(1, 0)
            fillers += qk_micros(1, 2) + qk_micros(4, 2)
            attention_block(1, 1)
            fillers += qk_micros(1, 3) + qk_micros(4, 3)
            attention_block(1, 2)
            fillers += qk_micros(2, 0) + qk_micros(5, 0)
            attention_block(1, 3)
            fillers += qk_micros(2, 1) + qk_micros(5, 1)
            attention_block(2, 0)
            fillers += qk_micros(2, 2) + qk_micros(5, 2)
            for ot in range(4):
                fillers += proj_micro(ot, 0)
            attention_block(2, 1)
            fillers += qk_micros(2, 3) + qk_micros(5, 3)
            fillers += proj_micro(4, 0) + proj_micro(5, 0)
            for ot in range(NCT):
                fillers += proj_micro(ot, 1)
            attention_block(2, 2)
            for ot in range(NCT):
                fillers += proj_micro(ot, 2)
            attention_block(2, 3)
            for ot in range(NCT):
                proj_micro(ot, 3)[0]()
    nc.compile()
    return nc


_CACHE: dict = {}


def _get_graph():
    if "nc" not in _CACHE:
        _CACHE["nc"] = _build_graph()
    return _CACHE["nc"]


def _bf16(a):
    return np.ascontiguousarray(a.astype(ml_dtypes.bfloat16))


def _pack(a):
    """[n*128, W] -> [128, n*W]: row ct*128+p, col w -> row p, col ct*W+w."""
    n = a.shape[0] // 128
    return a.reshape(n, 128, a.shape[1]).transpose(1, 0, 2).reshape(128, -1)


def _make_tri():
    k = np.arange(128)[:, None]
    q = np.arange(128)[None, :]
    pat = (q >= k).astype(np.float32)
    return _bf16(np.concatenate([pat, pat], axis=1))  # [128, 256], dup for c=2


def _prepare_in_maps(x, W_attn, b_attn, W_proj):
    tri = _make_tri()
    in_maps = []
    for core in range(NCORES):
        b, hh = core // 2, core % 2
        sl = slice(hh * CH, (hh + 1) * CH)
        wq = W_attn[0 * C:1 * C][sl]          # [384, 768]
        wk = W_attn[1 * C:2 * C][sl]
        wv = W_attn[2 * C:3 * C][sl]
        bq = b_attn[0 * C:1 * C][sl]
        bk = b_attn[1 * C:2 * C][sl]
        in_maps.append({
            "xT": _bf16(_pack(x[b].T)),                            # [128, 6*2048]
            "wqkT": _bf16(_pack(np.concatenate([wq, wk], 0).T)),   # [128, 6*768]
            "bqk": np.ascontiguousarray(
                np.concatenate([bq, bk]).reshape(-1, 128).T),      # [128, 6]
            "wvT": _bf16(_pack(wv.T)),                             # [128, 6*384]
            "wpT": _bf16(_pack(W_proj[:, sl].T)),                  # [128, 3*768]
            "tri": tri,
        })
    return in_maps


def _unshard(outs, W_proj, b_attn, b_proj):
    bv = b_attn[2 * C:3 * C]
    const = W_proj @ bv + b_proj                     # [768]
    y = np.empty((B, T, C), np.float32)
    for b in range(B):
        acc = (np.asarray(outs[2 * b]["out"], np.float32)
               + np.asarray(outs[2 * b + 1]["out"], np.float32))
        y[b] = acc.T + const
    return y


def run(x, W_attn, b_attn, W_proj, b_proj, **spmd_kwargs):
    x = np.asarray(x, np.float32)
    W_attn = np.asarray(W_attn, np.float32)
    b_attn = np.asarray(b_attn, np.float32)
    W_proj = np.asarray(W_proj, np.float32)
    b_proj = np.asarray(b_proj, np.float32)
    in_maps = _prepare_in_maps(x, W_attn, b_attn, W_proj)
    nc = _get_graph()
    res = run_bass_kernel_spmd(
        nc, in_maps, core_ids=list(range(NCORES)), **spmd_kwargs
    )
    return _unshard(res.results, W_proj, b_attn, b_proj), res


def kernel(x, W_attn, b_attn, W_proj, b_proj):
    y, _ = run(x, W_attn, b_attn, W_proj, b_proj)
    return y
